# revision 40
# baseline (speedup 1.0000x reference)
"""Trainium2 Bass kernel v3 for nn_CrossAttention (8-core data-parallel over batch).

Math (per batch b = one NeuronCore):
  x1 = x + PEx ; y1 = y + PEy           (raw-reshape positional encodings)
  q  = conv3x3(relu(conv3x3(x1,wq1)+bq1), wq2*s)+bq2*s   viewed as (1024,128)
  k  = conv3x3(relu(conv3x3(y1,wk1)+bk1), wk2)+bk2       viewed as (4096,128)
  out = softmax(q @ k.T) @ z.flat       (s = 1/sqrt(128) folded into wq2/bq2)

v3 design vs v2:
  - kT groups 2..7 are transposed by the DMA xbar engines
    (dma_start_transpose, one [C,512]->[pix,blk,cout] call per group) instead
    of PE transposes + DVE/Act copies: frees ~1.3us PE + ~4us DVE/Act. The
    head-critical groups (q0,q1,k0,k1) keep the low-latency PE-transpose path
    (DMA transpose has ~2.3us fixed latency).
  - minimal critical chain to the first exp: qc1 x2, kc1 0-1, qc2(0), kc2(0),
    then the negmax prepass; remaining conv work rides as filler inside the
    attention stream (spread tuned per chunk: [5,4,4,1]).
  - last (3,7) slot split into 2x512 halves -> shorter end drain.
  - logits pre-scaled by A=128/ln2 (folded into wq2) so a Schraudolph
    fast-exp on DVE is a single tensor_scalar (optional S_SLOTS, off by
    default; Act exp uses scale=1/A).
  - PSUM plan: conv1 2 banks + conv2/pt 2 banks + psl 4 banks = 8 exactly.
  - NOTE hardware ISA constraints (walrus-verified): matmul stationary APs
    must have ONE free dim (no swapped-conv trick), and Pool/GpSimd cannot
    run TensorScalarPtr nor touch PSUM - numerator stays on DVE.
"""

import numpy as np
import ml_dtypes

import concourse.bass as bass
import concourse.mybir as mybir
import concourse.tile as tile
from concourse import bacc
from concourse.bass import ts
from concourse.bass_utils import run_bass_kernel_spmd

F32 = mybir.dt.float32
BF16 = mybir.dt.bfloat16
AF = mybir.ActivationFunctionType
ALU = mybir.AluOpType

C = 128
A = 32          # q spatial side
H = 64          # k spatial side
SQ = A * A      # 1024
SK = H * H      # 4096
SCALE = float(C ** -0.5)
A_EXP = float(128.0 / np.log(2.0))   # Schraudolph bf16 scale
N_CORES = 8
NPBF = ml_dtypes.bfloat16


def _make_pe(dim, length):
    pos = np.arange(length, dtype=np.float32)[:, None]
    div = np.exp(np.arange(0, dim, 2, dtype=np.float32) * np.float32(-np.log(10000.0) / dim))
    pe = np.zeros((length, dim), dtype=np.float32)
    pe[:, 0::2] = np.sin(pos * div)
    pe[:, 1::2] = np.cos(pos * div)
    return pe


def _build_program(repeat=1, staggered=False):
    nc = bacc.Bacc("TRN2", target_bir_lowering=False, debug=False, num_devices=N_CORES)

    dx = nc.dram_tensor("x", [C, SQ], BF16, kind="ExternalInput")
    dy = nc.dram_tensor("y", [C, SK], BF16, kind="ExternalInput")
    dv = nc.dram_tensor("vz", [1, SK], BF16, kind="ExternalInput")
    dw = {n: nc.dram_tensor(n, [C, 9 * C], BF16, kind="ExternalInput")
          for n in ("wq1", "wq2", "wk1", "wk2")}
    db = {n: nc.dram_tensor(n, [C, 1], F32, kind="ExternalInput")
          for n in ("bq1", "bk1", "bq2", "bk2")}
    dident = nc.dram_tensor("ident", [C, C], BF16, kind="ExternalInput")
    dpex = nc.dram_tensor("pex", [C, SQ], BF16, kind="ExternalInput")
    dpey = nc.dram_tensor("pey", [C, SK], BF16, kind="ExternalInput")
    dout = nc.dram_tensor("out", [SQ, 1], F32, kind="ExternalOutput")

    XP, YP = A + 2, H + 2          # padded sides: 34, 66
    with tile.TileContext(nc) as tc:
        with (
            tc.tile_pool(name="const", bufs=1) as cst,
            tc.tile_pool(name="pp", bufs=3) as ppool,
            tc.tile_pool(name="kimg", bufs=3) as kip,
            tc.tile_pool(name="scr", bufs=3) as scrp,
            tc.tile_pool(name="psc", bufs=2, space="PSUM") as psc,
            tc.tile_pool(name="psd", bufs=2, space="PSUM") as psd,
            tc.tile_pool(name="psa", bufs=2, space="PSUM") as psa,
        ):
          import contextlib
          loop_cm = (tc.For_i(0, repeat, 1,
                              hint_engines=(mybir.EngineType.PE, mybir.EngineType.Activation,
                                            mybir.EngineType.DVE, mybir.EngineType.SP,
                                            mybir.EngineType.Pool),
                              staggered_reset=staggered)
                     if repeat > 1 else contextlib.nullcontext())
          # ---- once-only prologue: PE warm-up ramps the pstate while the
          # first DMAs land; Exp table preload; constant zero borders ----
          wmem = cst.tile([C, 512], BF16, tag="wmem")
          nc.gpsimd.memset(wmem[:], 0.0)
          wps = psa.tile([C, 1024], F32, tag="psl", name="wps")
          for i in range(8):
              nc.tensor.matmul(wps[:, 0:512], wmem[:, 0:C], wmem[:],
                               start=True, stop=True)
          wexp = cst.tile([C, 1], BF16, tag="wexp")
          nc.scalar.activation(wexp[:], wmem[:, 0:1], AF.Exp)

          zrow = cst.tile([C, YP], BF16, tag="zrow")
          nc.gpsimd.memset(zrow[:], 0.0)
          ones_rep = cst.tile([C, 1024], BF16, tag="ones_rep")
          nc.gpsimd.memset(ones_rep[:], 1.0)

          def pad_tile(tag, side):
              t = cst.tile([C, side * side], BF16, tag=tag, name=tag)
              t3 = t[:].rearrange("p (r c) -> p r c", c=side)
              zr = zrow[:, 0:side].rearrange("p (a c) -> p a c", a=1)
              zc = zrow[:, 0:side - 2].rearrange("p (r a) -> p r a", a=1)
              nc.gpsimd.tensor_copy(t3[:, 0:1, :], zr)
              nc.gpsimd.tensor_copy(t3[:, side - 1:side, :], zr)
              nc.gpsimd.tensor_copy(t3[:, 1:side - 1, 0:1], zc)
              nc.gpsimd.tensor_copy(t3[:, 1:side - 1, side - 1:side], zc)
              return t

          x_pad = pad_tile("x_pad", XP)
          t1q = pad_tile("t1q", XP)
          y_pad = pad_tile("y_pad", YP)
          t1k = pad_tile("t1k", YP)

          with loop_cm:
            # ---- DMA loads (q-path critical chain first) ----
            x_raw = cst.tile([C, SQ], BF16, tag="x_raw")
            pex = cst.tile([C, SQ], BF16, tag="pex")
            w_sb, b_sb = {}, {}

            def load_w(n):
                w_sb[n] = cst.tile([C, 9 * C], BF16, tag=n, name=n + "_sb")
                for h in range(2):
                    nc.sync.dma_start(out=w_sb[n][:, ts(h, 576)], in_=dw[n].ap()[:, ts(h, 576)])

            def load_b(n):
                b_sb[n] = cst.tile([C, 1], F32, tag=n, name=n + "_sb")
                nc.sync.dma_start(out=b_sb[n][:], in_=db[n].ap())

            nc.sync.dma_start(out=x_raw[:, ts(0, SQ // 2)], in_=dx.ap()[:, ts(0, SQ // 2)])
            load_w("wq1")
            nc.sync.dma_start(out=pex[:, ts(0, SQ // 2)], in_=dpex.ap()[:, ts(0, SQ // 2)])
            load_b("bq1")
            nc.sync.dma_start(out=x_raw[:, ts(1, SQ // 2)], in_=dx.ap()[:, ts(1, SQ // 2)])
            nc.sync.dma_start(out=pex[:, ts(1, SQ // 2)], in_=dpex.ap()[:, ts(1, SQ // 2)])

            y_raw = cst.tile([C, SK], BF16, tag="y_raw")
            pey = cst.tile([C, SK], BF16, tag="pey")
            v_rep = cst.tile([C, SK], BF16, tag="v_rep")

            def load_y(h):
                nc.sync.dma_start(out=y_raw[:, ts(h, SK // 4)], in_=dy.ap()[:, ts(h, SK // 4)])
                nc.sync.dma_start(out=pey[:, ts(h, SK // 4)], in_=dpey.ap()[:, ts(h, SK // 4)])

            def load_v(h):
                nc.sync.dma_start(out=v_rep[:, ts(h, SK // 4)],
                                  in_=dv.ap()[:, ts(h, SK // 4)].broadcast_to((C, SK // 4)))

            load_y(0)
            load_w("wk1"); load_b("bk1")
            load_y(1)
            load_w("wq2"); load_b("bq2")
            ident = cst.tile([C, C], BF16, tag="ident")
            nc.sync.dma_start(out=ident[:], in_=dident.ap())
            load_w("wk2"); load_b("bk2")
            load_y(2)
            load_v(0)
            load_y(3)
            load_v(1); load_v(2); load_v(3)

            x_pad3 = x_pad[:].rearrange("p (r c) -> p r c", c=XP)
            t1q3 = t1q[:].rearrange("p (r c) -> p r c", c=XP)
            y_pad3 = y_pad[:].rearrange("p (r c) -> p r c", c=YP)
            t1k3 = t1k[:].rearrange("p (r c) -> p r c", c=YP)

            # x1 = x + PEx into padded interior (DVE 2x); halves so the first
            # conv taps start as soon as piece 0 lands
            for h in range(2):
                nc.vector.tensor_tensor(
                    out=x_pad3[:, 16 * h + 1:16 * h + 17, 1:A + 1],
                    in0=x_raw[:, ts(h, SQ // 2)].rearrange("p (r c) -> p r c", c=A),
                    in1=pex[:, ts(h, SQ // 2)].rearrange("p (r c) -> p r c", c=A),
                    op=ALU.add)

            def y1_add(h):
                nc.vector.tensor_tensor(
                    out=y_pad3[:, 16 * h + 1:16 * h + 17, 1:H + 1],
                    in0=y_raw[:, ts(h, SK // 4)].rearrange("p (r c) -> p r c", c=H),
                    in1=pey[:, ts(h, SK // 4)].rearrange("p (r c) -> p r c", c=H),
                    op=ALU.add)

            y1_add(0)

            # ---- conv1 (standard orientation: [cout, pix] psum) ----
            def conv_mms(src3, w, rows0, nrows, side_c, ps, i0=0, i1=9):
                ps3 = ps[:].rearrange("p (r c) -> p r c", c=side_c)
                i = 0
                for dyy in range(3):
                    for dxx in range(3):
                        if i0 <= i < i1:
                            rhs = src3[:, rows0 + dyy: rows0 + dyy + nrows,
                                       dxx: dxx + side_c]
                            nc.tensor.matmul(ps3, w[:, ts(i, C)], rhs,
                                             start=(i == 0), stop=(i == 8))
                        i += 1

            def q_conv1(n):
                ps1 = psc.tile([C, 512], F32, tag="cps", name=f"qps1_{n}")
                conv_mms(x_pad3, w_sb["wq1"], 16 * n, 16, A, ps1)
                nc.scalar.activation(
                    t1q3[:, 16 * n + 1:16 * n + 17, 1:A + 1],
                    ps1[:].rearrange("p (r c) -> p r c", c=A),
                    AF.Relu, bias=b_sb["bq1"][:])

            def conv1_k_ops(t):
                """Closures: 3 matmul groups + epilogue for k conv1 tile t."""
                ops = []
                ps_box = []
                def mk(i0, i1):
                    def f():
                        if not ps_box:
                            ps_box.append(psc.tile([C, 512], F32, tag="cps", name=f"cps_{t}"))
                        conv_mms(y_pad3, w_sb["wk1"], 8 * t, 8, H, ps_box[0], i0, i1)
                    return f
                for (i0, i1) in ((0, 3), (3, 6), (6, 9)):
                    ops.append(mk(i0, i1))
                def epi():
                    if t <= 1:   # Act is idle before the exp stream
                        nc.scalar.activation(
                            t1k3[:, 8 * t + 1:8 * t + 9, 1:H + 1],
                            ps_box[0][:].rearrange("p (r c) -> p r c", c=H),
                            AF.Relu, bias=b_sb["bk1"][:])
                    else:
                        nc.vector.tensor_scalar(
                            out=t1k3[:, 8 * t + 1:8 * t + 9, 1:H + 1],
                            in0=ps_box[0][:].rearrange("p (r c) -> p r c", c=H),
                            scalar1=b_sb["bk1"][:], scalar2=0.0, op0=ALU.add, op1=ALU.max)
                ops.append(epi)
                return ops

            # ---- conv2 (swapped: stationary = input window, psum = [pix, cout]
            # = attention layout). Group g = 4 blocks = 512 psum cols. ----
            qT = cst.tile([C, SQ], BF16, tag="qT")
            kT = cst.tile([C, SK], BF16, tag="kT")

            def conv2_q_ops(g):
                """q conv2 tile g (512 px, standard orient) + DMA-transpose
                into qT. Epilogue bias-add on Act (idle pre-exp-stream)."""
                ops = []
                ps_box = []
                def mk(i0, i1):
                    def f():
                        if not ps_box:
                            ps_box.append(psd.tile([C, 512], F32, tag="c2ps", name=f"qc2_{g}"))
                        conv_mms(t1q3, w_sb["wq2"], 16 * g, 16, A, ps_box[0], i0, i1)
                    return f
                for (i0, i1) in ((0, 3), (3, 6), (6, 9)):
                    ops.append(mk(i0, i1))
                def epi():
                    img = kip.tile([C, 512], BF16, tag="kimg", name=f"qimg_{g}")
                    nc.scalar.activation(img[:], ps_box[0][:], AF.Identity,
                                         bias=b_sb["bq2"][:])
                    pt = psd.tile([C, 512], BF16, tag="c2ps", name=f"ptq_{g}")
                    for i in range(4):
                        nc.tensor.transpose(pt[:, ts(i, C)], img[:, ts(i, C)], ident[:])
                    nc.vector.tensor_copy(qT[:, ts(g, 512)], pt[:])
                ops.append(epi)
                return ops

            def conv2_k_ops(g):
                """k conv2 tile g (512 px) + DMA-transpose into kT."""
                ops = []
                ps_box = []
                def mk(i0, i1):
                    def f():
                        if not ps_box:
                            ps_box.append(psd.tile([C, 512], F32, tag="c2ps", name=f"kc2_{g}"))
                        conv_mms(t1k3, w_sb["wk2"], 8 * g, 8, H, ps_box[0], i0, i1)
                    return f
                for (i0, i1) in ((0, 3), (3, 6), (6, 9)):
                    ops.append(mk(i0, i1))
                def epi():
                    img = kip.tile([C, 512], BF16, tag="kimg", name=f"kimg_{g}")
                    if g <= 1:   # head-critical: low-latency PE transpose
                        nc.scalar.activation(img[:], ps_box[0][:], AF.Identity,
                                             bias=b_sb["bk2"][:])
                        pt = psd.tile([C, 512], BF16, tag="c2ps", name=f"ptk_{g}")
                        for i in range(4):
                            nc.tensor.transpose(pt[:, ts(i, C)], img[:, ts(i, C)],
                                                ident[:])
                        nc.vector.tensor_copy(kT[:, ts(g, 512)], pt[:])
                    else:        # off critical path: free DMA-engine transpose
                        nc.vector.tensor_scalar(
                            out=img[:], in0=ps_box[0][:],
                            scalar1=b_sb["bk2"][:], scalar2=None, op0=ALU.add)
                        nc.sync.dma_start_transpose(
                            out=kT[:, ts(g, 512)].rearrange("p (b c) -> p b c", c=C),
                            in_=img[:])
                ops.append(epi)
                return ops

            # ---- attention state ----
            # 4 key chunks of 1024; col = 4*m + c in dacc/nacc.
            # qT/psl carry logits pre-scaled by A_C = 128/ln2 (folded into the
            # wq2 weights host-side) so the Schraudolph fast-exp on DVE fits
            # in a single tensor_scalar: bits(bf16 e^u) ~ int16(A*u + B).
            CH = [(0, 1024), (1024, 1024), (2048, 1024), (3072, 1024)]
            NCH = len(CH)
            A_C = A_EXP
            B_C = float(127.0 * 128.0 - 7.5)
            negM = cst.tile([C, 8], F32, tag="negM")    # -A*rowmax (subsample)
            negMa = cst.tile([C, 8], F32, tag="negMa")  # -rowmax (for Act exp)
            Bp = cst.tile([C, 8], F32, tag="Bp")        # B_C + negM (for DVE)
            dacc = cst.tile([C, 33], F32, tag="dacc")   # col 32: (3,7) half-b
            nacc = cst.tile([C, 33], F32, tag="nacc")

            # per-row softmax shifts from the stride-32 key subsample (kT cols
            # 0:128); exact after renorm since gap to true rowmax << exp range
            prepass_box = {}

            def negmax_prepass_mms(half):
                sub = psc.tile([C, 512], F32, tag="cps", name=f"pslsub{half}")
                for j in range(4):
                    m = 4 * half + j
                    nc.tensor.matmul(sub[:, ts(j, C)], qT[:, ts(m, C)],
                                     kT[:, 0:C], start=True, stop=True)
                prepass_box[half] = sub

            def negmax_prepass_reduce(half):
                sub = prepass_box[half]
                if half == 0:
                    # m=0 alone first: it gates exp(0,0)
                    nc.vector.tensor_reduce(
                        out=negM[:, 0:1], in_=sub[:, 0:C],
                        axis=mybir.AxisListType.X, op=ALU.max, negate=True)
                    nc.vector.tensor_reduce(
                        out=negM[:, 1:4],
                        in_=sub[:, C:].rearrange("p (m k) -> p m k", k=C),
                        axis=mybir.AxisListType.X, op=ALU.max, negate=True)
                else:
                    nc.vector.tensor_reduce(
                        out=negM[:, 4:8],
                        in_=sub[:].rearrange("p (m k) -> p m k", k=C),
                        axis=mybir.AxisListType.X, op=ALU.max, negate=True)
                ms = slice(4 * half, 4 * half + 4)
                nc.vector.tensor_scalar(out=negMa[:, ms], in0=negM[:, ms],
                                        scalar1=1.0 / A_C, scalar2=None, op0=ALU.mult)
                nc.vector.tensor_scalar(out=Bp[:, ms], in0=negM[:, ms],
                                        scalar1=B_C, scalar2=None, op0=ALU.add)

            import os
            S_SLOTS = set()
            for tok in os.environ.get("S_SLOTS", "").split(","):
                if tok:
                    cc, mm = tok.split(".")
                    S_SLOTS.add((int(cc), int(mm)))
            def _slotset(env, dflt):
                out = set()
                for tok in os.environ.get(env, dflt).split(","):
                    if tok:
                        cc, mm = tok.split(".")
                        out.add((int(cc), int(mm)))
                return out
            N_DVE = _slotset("N_DVE", "")
            D_DVE = _slotset("D_DVE", "2.7,3.1,3.3,3.5")

            def att_ops(c, m, off=None, sz=None, colc=None):
                if off is None:
                    off, sz = CH[c]
                colc = c if colc is None else colc
                pslf = psa.tile([C, 1024], F32, tag="psl", name=f"psl_{c}_{m}")
                psl = pslf[:, 0:sz]
                for u in range(sz // 512):   # a matmul cannot span PSUM banks
                    nc.tensor.matmul(psl[:, ts(u, 512)], qT[:, ts(m, C)],
                                     kT[:, off + 512 * u: off + 512 * (u + 1)],
                                     start=True, stop=True)
                Pf = ppool.tile([C, 1024], BF16, tag="P", name=f"P_{c}_{m}")
                P = Pf[:, 0:sz]
                col = NCH * m + colc
                if (c, m) in S_SLOTS:
                    # Schraudolph fast-exp on DVE: int16(max(psl + Bp, 0))
                    # bitcast as bf16 ~= exp((psl/A) - M) to ~2-4% per weight
                    nc.vector.tensor_scalar(
                        out=Pf[:, 0:sz].bitcast(mybir.dt.int16), in0=psl,
                        scalar1=Bp[:, m:m + 1], scalar2=0.0,
                        op0=ALU.add, op1=ALU.max)
                    # denominator on Pool (Act is the scarce engine here):
                    # stt P*1 with accum (gpsimd reduce is partition-axis only)
                    nc.vector.tensor_reduce(
                        out=dacc[:, col:col + 1], in_=P,
                        axis=mybir.AxisListType.X, op=ALU.add)
                else:
                    nc.scalar.activation(P, psl, AF.Exp, bias=negMa[:, m:m + 1],
                                         scale=1.0 / A_C,
                                         accum_out=dacc[:, col:col + 1])
                scrapf = scrp.tile([C, 1024], BF16, tag="scrap", name=f"scrap_{c}_{m}")
                neng = nc.vector
                neng.scalar_tensor_tensor(out=scrapf[:, 0:sz], in0=P, scalar=1.0,
                                          in1=v_rep[:, off:off + sz],
                                          op0=ALU.bypass, op1=ALU.mult,
                                          accum_out=nacc[:, col:col + 1])

            # ---- emission: minimal chain to the first exp ----
            q_conv1(0)
            q_conv1(1)
            y1_add(1)         # kc1 tile 1 reads padded row 17 (quarter 1)
            for op in conv1_k_ops(0):
                op()
            for op in conv2_q_ops(0):
                op()
            for op in conv1_k_ops(1):
                op()
            for op in conv2_k_ops(0):
                op()
            for op in conv1_k_ops(2):
                op()
            y1_add(2)         # kc1 tile 3 reads padded row 33 (quarter 2)
            for op in conv2_k_ops(1):
                op()
            negmax_prepass_mms(0)
            negmax_prepass_reduce(0)
            for op in conv1_k_ops(3):
                op()

            dsum = cst.tile([C, 8], F32, tag="dsum")
            nsum = cst.tile([C, 8], F32, tag="nsum")
            recip = cst.tile([C, 8], F32, tag="recip")
            res = cst.tile([C, 8], F32, tag="res")
            dout_v = dout.ap().rearrange("(co m) one -> co (m one)", m=8)

            def finale_half(h):
                ms = slice(4 * h, 4 * h + 4)
                cs = slice(4 * NCH * h, 4 * NCH * h + 4 * NCH)
                nc.vector.tensor_reduce(
                    out=dsum[:, ms], in_=dacc[:, cs].rearrange("p (m c) -> p m c", c=NCH),
                    axis=mybir.AxisListType.X, op=ALU.add)
                nc.vector.tensor_reduce(
                    out=nsum[:, ms], in_=nacc[:, cs].rearrange("p (m c) -> p m c", c=NCH),
                    axis=mybir.AxisListType.X, op=ALU.add)
                if h == 1:
                    # (3,7) was split; fold its second half (col 32) into m=7
                    nc.vector.tensor_tensor(out=dsum[:, 7:8], in0=dsum[:, 7:8],
                                            in1=dacc[:, 32:33], op=ALU.add)
                    nc.vector.tensor_tensor(out=nsum[:, 7:8], in0=nsum[:, 7:8],
                                            in1=nacc[:, 32:33], op=ALU.add)
                nc.vector.reciprocal(recip[:, ms], dsum[:, ms])
                nc.vector.tensor_tensor(out=res[:, ms], in0=nsum[:, ms],
                                        in1=recip[:, ms], op=ALU.mult)
                nc.sync.dma_start(out=dout_v[:, ms], in_=res[:, ms])

            # conv filler, spread across the attention stream so PE never
            # starves. Chunk c logits need kT groups {2c, 2c+1};
            # qT grp 1 + negM half 1 before m=4 of chunk 0.
            conv_q = []
            conv_q.extend(conv2_q_ops(1))
            conv_q.append(lambda: negmax_prepass_mms(1))
            conv_q.append(lambda: negmax_prepass_reduce(1))
            conv_q.extend(conv2_k_ops(2))      # kT2
            conv_q.extend(conv1_k_ops(4))
            conv_q.extend(conv2_k_ops(3))      # kT3
            cutA = len(conv_q)                 # chunk 1 logits need kT2,kT3
            conv_q.append(lambda: y1_add(3))   # kc1 tile 5 reads quarter 3
            conv_q.extend(conv1_k_ops(5))
            conv_q.extend(conv2_k_ops(4))      # kT4
            conv_q.extend(conv1_k_ops(6))
            conv_q.extend(conv2_k_ops(5))      # kT5
            cutB = len(conv_q)                 # chunk 2 logits need kT4,kT5
            conv_q.extend(conv1_k_ops(7))
            conv_q.extend(conv2_k_ops(6))      # kT6
            conv_q.extend(conv2_k_ops(7))      # kT7
            n_ops = len(conv_q)
            cuts = {0: cutA, 1: cutB, 2: n_ops, 3: n_ops}
            if os.environ.get("SCHED_SPREAD"):
                spread = [int(s) for s in os.environ["SCHED_SPREAD"].split(",")]
            else:
                spread = [5, 4, 4, 1]
            pos = 0
            for cchunk in range(NCH):
                if cchunk == NCH - 1:
                    # last chunk's logits read kT7: all remaining filler first
                    for op in conv_q[pos:]:
                        op()
                    pos = n_ops
                hi = cuts[cchunk]
                navail = hi - pos
                ns = max(spread[cchunk], 1)
                for m in range(8):
                    if cchunk == 3 and m == 7:
                        att_ops(3, 7, off=3072, sz=512)
                        att_ops(3, 7, off=3584, sz=512, colc=4)
                    else:
                        att_ops(cchunk, m)
                    k0 = pos + min(navail, (navail * m) // ns)
                    k1 = pos + min(navail, (navail * (m + 1)) // ns)
                    for op in conv_q[k0:k1]:
                        op()
                    if cchunk == NCH - 1 and m == 3:
                        finale_half(0)
                pos = hi
            finale_half(1)

    nc.compile()
    return nc


_NC_CACHE = []


def _host_prep(x, y, z, wq1, bq1, wq2, bq2, wk1, bk1, wk2, bk2):
    B = x.shape[0]
    wmap = {}
    for name, w, s in (("wq1", wq1, 1.0), ("wq2", wq2, SCALE * A_EXP),
                       ("wk1", wk1, 1.0), ("wk2", wk2, 1.0)):
        wmap[name] = np.ascontiguousarray(
            (np.asarray(w, dtype=np.float32) * s).transpose(1, 2, 3, 0).reshape(C, 9 * C)
        ).astype(NPBF)
    bmap = {"bq1": bq1, "bq2": np.asarray(bq2, np.float32) * (SCALE * A_EXP),
            "bk1": bk1, "bk2": bk2}
    bmap = {n: np.ascontiguousarray(np.asarray(b, np.float32).reshape(C, 1))
            for n, b in bmap.items()}
    pex = np.ascontiguousarray(_make_pe(C, SQ).reshape(C, SQ)).astype(NPBF)
    ident = np.eye(C, dtype=np.float32).astype(NPBF)
    pey = np.ascontiguousarray(_make_pe(C, SK).reshape(C, SK)).astype(NPBF)
    # v in kT column order: col m*128+co  ->  z_flat[co*32+m]
    zperm = np.ascontiguousarray(
        np.asarray(z, np.float32).reshape(B, SK).reshape(B, C, SK // C)
        .transpose(0, 2, 1).reshape(B, 1, SK)).astype(NPBF)

    in_maps = []
    for b in range(B):
        m = {
            "x": np.ascontiguousarray(np.asarray(x, np.float32)[b].reshape(C, SQ)).astype(NPBF),
            "y": np.ascontiguousarray(np.asarray(y, np.float32)[b].reshape(C, SK)).astype(NPBF),
            "vz": zperm[b],
            "pex": pex, "pey": pey, "ident": ident,
        }
        m.update(wmap)
        m.update(bmap)
        in_maps.append(m)
    return in_maps


def kernel(x, y, z, wq1, bq1, wq2, bq2, wk1, bk1, wk2, bk2):
    x = np.asarray(x, dtype=np.float32)
    B = x.shape[0]
    assert B == N_CORES

    if not _NC_CACHE:
        _NC_CACHE.append(_build_program())
    nc = _NC_CACHE[0]

    in_maps = _host_prep(x, y, z, wq1, bq1, wq2, bq2, wk1, bk1, wk2, bk2)
    res = run_bass_kernel_spmd(nc, in_maps, core_ids=list(range(N_CORES)))
    out = np.stack([res.results[b]["out"].reshape(SQ, 1) for b in range(B)])
    return out.astype(np.float32)
